# revision 32
# baseline (speedup 1.0000x reference)
"""Causal BoW (running mean over T) Trainium2 kernel.

out[b, t, c] = sum_{s<=t} x[b, s, c] / (t+1)   for x of shape [32, 2048, 512] f32.

Sharding: batch B=32 across 8 NeuronCores (4 samples each), no cross-core comms.

Per-core algorithm (per sample [T=2048, C=512], 16 T-blocks of 128 rows):
  - f32 matmuls cost 4 cycles/row on the PE; float32r costs ~1 cycle/row but
    keeps only 11 mantissa bits. x is split on-chip into
    x_hi = round_f32r(x) (ACT copy) and x_lo = round_f32r(x - x_hi) (DVE sub);
    streaming both through the PE reconstructs full fp32 precision (verified
    bit-exact on HW) at ~2 cycles/row total.
  - Block scan: psum_j = U128^T.T @ xhi_j + U128^T.T @ xlo_j (U128 =
    upper-triangular ones). All scan matmuls share one weight matrix so the
    PE streams back-to-back (~240-330 ns per N=512 matmul).
  - Block offsets: accumulating matmuls with "step" selector weights
    (step_k[p, m] = 1 if m > k) produce off[m, c] = sum_{k<m} tot_k in one
    PSUM bank; split into off_hi/off_lo f32r rows.
  - Offset broadcast: off_hi[j] / off_lo[j] are scattered by two tiny
    SBUF->SBUF DMAs into partitions 0/1 of a per-sample staging tile (DMA
    moves data across partitions freely; compute engines cannot), then
    psum_j += ones2^T.T @ bo[:, j] — a K=2 matmul whose all-ones [2,128]
    weight is shared by every block, avoiding per-block weight reloads.
  - Eviction: Copy with per-partition scale recip[p, j] = 1/(j*128+p+1)
    applied while moving PSUM -> SBUF, alternating ACT/DVE.
  - All DMAs keep full 128-partition access patterns: odd partition counts
    (e.g. 127) defeat the HW-DGE multi-engine fanout and serialize all
    traffic onto one DMA engine (measured 7x regression).
"""

import numpy as np

import concourse.bass as bass
import concourse.bacc as bacc
import concourse.mybir as mybir
from concourse import tile
from concourse.bass_utils import run_bass_kernel_spmd

B, T, C = 32, 2048, 512
N_CORES = 8
BS = B // N_CORES          # samples per core
P = 128                    # partitions / T-block size
NBLK = T // P              # 16 blocks per sample
NQ = 4                     # tile groups per sample
NH = NBLK // NQ            # blocks per tile group (4)
F32 = mybir.dt.float32
F32R = mybir.dt.float32r

_cache = {}


def _build():
    nc = bacc.Bacc()
    x = nc.dram_tensor("x", [BS, T, C], F32, kind="ExternalInput")
    u128 = nc.dram_tensor("u128", [P, P], F32R, kind="ExternalInput")
    stepm = nc.dram_tensor("stepm", [P, NBLK * NBLK], F32R, kind="ExternalInput")
    ones2 = nc.dram_tensor("ones2", [2, P], F32R, kind="ExternalInput")
    recip = nc.dram_tensor("recip", [P, NBLK], F32, kind="ExternalInput")
    y = nc.dram_tensor("y", [BS, T, C], F32, kind="ExternalOutput")

    HALF = NH * C

    with tile.TileContext(nc) as tc:
        with (
            tc.tile_pool(name="singles", bufs=1) as singles,
            tc.tile_pool(name="xp", bufs=3) as xpool,
            tc.tile_pool(name="xhp", bufs=5) as xhpool,
            tc.tile_pool(name="xlp", bufs=5) as xlpool,
            tc.tile_pool(name="op", bufs=4) as opool,
            tc.tile_pool(name="offp", bufs=2) as offpool,
            tc.tile_pool(name="bop", bufs=4) as bopool,
            tc.tile_pool(name="pblk", bufs=6, space="PSUM") as pblk,
            tc.tile_pool(name="poff", bufs=2, space="PSUM") as poff,
        ):
            u_t = singles.tile([P, P], F32R)
            nc.sync.dma_start(out=u_t[:], in_=u128[:])
            step_t = singles.tile([P, NBLK * NBLK], F32R)
            nc.sync.dma_start(out=step_t[:], in_=stepm[:])
            ones2_t = singles.tile([2, P], F32R)
            nc.sync.dma_start(out=ones2_t[:], in_=ones2[:])
            recip_t = singles.tile([P, NBLK], F32)
            nc.sync.dma_start(out=recip_t[:], in_=recip[:])

            for b in range(BS):
                xs = x[b].rearrange("(j p) c -> p j c", p=P)   # [128, 16, 512]
                ys = y[b].rearrange("(j p) c -> p j c", p=P)

                xhs, xls = [], []
                for h in range(NQ):
                    xt = xpool.tile([P, HALF], F32, tag="xt")
                    xt3 = xt.rearrange("p (j c) -> p j c", c=C)
                    nc.sync.dma_start(out=xt3[:], in_=xs[:, h * NH:(h + 1) * NH, :])
                    xh = xhpool.tile([P, HALF], F32R, tag="xh")
                    nc.scalar.copy(out=xh[:], in_=xt[:])
                    # lo-subtract on the otherwise-idle GPSIMD: keeps DVE free
                    # for evictions so the PE isn't starved of the next
                    # sample's splits at sample boundaries. (Rounding of the
                    # lo part is uncritical: |x_lo| <= 2^-12|x|, so even
                    # unrounded bits cost < 2^-24|x| at matmul ingest.)
                    xl = xlpool.tile([P, HALF], F32R, tag="xl")
                    nc.gpsimd.tensor_sub(out=xl[:], in0=xt[:], in1=xh[:].bitcast(F32))
                    xhs.append(xh)
                    xls.append(xl)

                # off[m, c] = sum_{k<m} (block-k column sum), one PSUM bank
                offp_t = poff.tile([NBLK, C], F32)
                for k in range(NBLK):
                    sel = step_t[:, k * NBLK:(k + 1) * NBLK]
                    for part, src in ((0, xhs), (1, xls)):
                        nc.tensor.matmul(
                            offp_t[:],
                            sel,
                            src[k // NH][:, (k % NH) * C:(k % NH + 1) * C],
                            start=(k == 0 and part == 0),
                            stop=(k == NBLK - 1 and part == 1),
                        )
                off_hi = offpool.tile([NBLK, C], F32R, tag="offhi")
                nc.scalar.copy(out=off_hi[:], in_=offp_t[:])
                off_lo = offpool.tile([NBLK, C], F32R, tag="offlo")
                nc.vector.tensor_sub(
                    out=off_lo[:], in0=offp_t[:], in1=off_hi[:].bitcast(F32)
                )

                # scatter offset rows to partitions 0/1 of per-quarter staging
                # tiles: bo[0, jj*C:(jj+1)*C] = off_hi[j], bo[1, ...] = off_lo[j]
                bos = []
                for h in range(NQ):
                    bo = bopool.tile([2, NH * C], F32R, tag="bo")
                    bo3 = bo.rearrange("p (j c) -> p j c", c=C)
                    nc.sync.dma_start(
                        out=bo3[0:1, :, :], in_=off_hi[h * NH:(h + 1) * NH, :]
                    )
                    nc.sync.dma_start(
                        out=bo3[1:2, :, :], in_=off_lo[h * NH:(h + 1) * NH, :]
                    )
                    bos.append(bo)

                # main scan: every matmul's weights are either U or ones2;
                # evictions all on DVE (ACT reads PSUM at ~half DVE's rate,
                # stretching the window in which PE matmuls contend with
                # eviction reads for PSUM bandwidth)
                for h in range(NQ):
                    ot = opool.tile([P, HALF], F32, tag="ot")
                    for jj in range(NH):
                        j = h * NH + jj
                        cs = slice(jj * C, (jj + 1) * C)
                        pb = pblk.tile([P, C], F32)
                        nc.tensor.matmul(pb[:], u_t[:], xhs[h][:, cs],
                                         start=True, stop=False)
                        nc.tensor.matmul(pb[:], u_t[:], xls[h][:, cs],
                                         start=False, stop=(j == 0))
                        if j > 0:
                            nc.tensor.matmul(
                                pb[:], ones2_t[:],
                                bos[h][:, jj * C:(jj + 1) * C],
                                start=False, stop=True,
                            )
                        nc.vector.tensor_scalar_mul(
                            ot[:, cs], pb[:], recip_t[:, j:j + 1]
                        )
                    ot3 = ot.rearrange("p (j c) -> p j c", c=C)
                    nc.sync.dma_start(
                        out=ys[:, h * NH:(h + 1) * NH, :], in_=ot3[:]
                    )
    nc.finalize()
    return nc


def _consts():
    u = np.triu(np.ones((P, P), dtype=np.float32))
    step = np.zeros((P, NBLK * NBLK), dtype=np.float32)
    for k in range(NBLK):
        for m in range(NBLK):
            if m > k:
                step[:, k * NBLK + m] = 1.0
    ones2 = np.ones((2, P), dtype=np.float32)
    recip = (1.0 / np.arange(1, T + 1, dtype=np.float32)).reshape(NBLK, P).T.copy()
    return u, step, ones2, recip


def run(x, trace=False):
    x = np.ascontiguousarray(np.asarray(x, dtype=np.float32))
    assert x.shape == (B, T, C), x.shape
    if "nc" not in _cache:
        _cache["nc"] = _build()
    nc = _cache["nc"]
    u, step, ones2, recip = _consts()
    in_maps = [
        {
            "x": np.ascontiguousarray(x[i * BS:(i + 1) * BS]),
            "u128": u,
            "stepm": step,
            "ones2": ones2,
            "recip": recip,
        }
        for i in range(N_CORES)
    ]
    res = run_bass_kernel_spmd(nc, in_maps, list(range(N_CORES)), trace=trace)
    y = np.concatenate([res.results[i]["y"] for i in range(N_CORES)], axis=0)
    return y, res.exec_time_ns


def kernel(x):
    y, _ = run(x, trace=False)
    return y


# revision 33
# speedup vs baseline: 1.5097x; 1.5097x over previous
"""Causal BoW (running mean over T) Trainium2 kernel.

out[b, t, c] = sum_{s<=t} x[b, s, c] / (t+1)   for x of shape [32, 2048, 512] f32.

Sharding: batch B=32 across 8 NeuronCores (4 samples each), no cross-core comms.

Per-core algorithm (per sample [T=2048, C=512], 16 T-blocks of 128 rows):
  - f32 matmuls cost 4 cycles/row on the PE; float32r costs ~1 cycle/row but
    keeps only 11 mantissa bits. x is split on-chip into
    x_hi = round_f32r(x) (ACT copy) and x_lo = round_f32r(x - x_hi) (DVE sub);
    streaming both through the PE reconstructs full fp32 precision (verified
    bit-exact on HW) at ~2 cycles/row total.
  - Block scan: psum_j = U128^T.T @ xhi_j + U128^T.T @ xlo_j (U128 =
    upper-triangular ones). All scan matmuls share one weight matrix so the
    PE streams back-to-back (~240-330 ns per N=512 matmul).
  - Block offsets: accumulating matmuls with "step" selector weights
    (step_k[p, m] = 1 if m > k) produce off[m, c] = sum_{k<m} tot_k in one
    PSUM bank; split into off_hi/off_lo f32r rows.
  - Offset broadcast: off_hi[j] / off_lo[j] are scattered by two tiny
    SBUF->SBUF DMAs into partitions 0/1 of a per-sample staging tile (DMA
    moves data across partitions freely; compute engines cannot), then
    psum_j += ones2^T.T @ bo[:, j] — a K=2 matmul whose all-ones [2,128]
    weight is shared by every block, avoiding per-block weight reloads.
  - Eviction: Copy with per-partition scale recip[p, j] = 1/(j*128+p+1)
    applied while moving PSUM -> SBUF, alternating ACT/DVE.
  - All DMAs keep full 128-partition access patterns: odd partition counts
    (e.g. 127) defeat the HW-DGE multi-engine fanout and serialize all
    traffic onto one DMA engine (measured 7x regression).
"""

import numpy as np

import concourse.bass as bass
import concourse.bacc as bacc
import concourse.mybir as mybir
from concourse import tile
from concourse.bass_utils import run_bass_kernel_spmd

B, T, C = 32, 2048, 512
N_CORES = 8
BS = B // N_CORES          # samples per core
P = 128                    # partitions / T-block size
NBLK = T // P              # 16 blocks per sample
NQ = 4                     # tile groups per sample
NH = NBLK // NQ            # blocks per tile group (4)
F32 = mybir.dt.float32
F32R = mybir.dt.float32r

_cache = {}


def _build():
    nc = bacc.Bacc()
    x = nc.dram_tensor("x", [BS, T, C], F32, kind="ExternalInput")
    u128 = nc.dram_tensor("u128", [P, P], F32R, kind="ExternalInput")
    stepm = nc.dram_tensor("stepm", [P, NBLK * NBLK], F32R, kind="ExternalInput")
    ones2 = nc.dram_tensor("ones2", [2, P], F32R, kind="ExternalInput")
    recip = nc.dram_tensor("recip", [P, NBLK], F32, kind="ExternalInput")
    y = nc.dram_tensor("y", [BS, T, C], F32, kind="ExternalOutput")

    HALF = NH * C

    with tile.TileContext(nc) as tc:
        with (
            tc.tile_pool(name="singles", bufs=1) as singles,
            tc.tile_pool(name="xp", bufs=3) as xpool,
            tc.tile_pool(name="xhp", bufs=5) as xhpool,
            tc.tile_pool(name="xlp", bufs=5) as xlpool,
            tc.tile_pool(name="op", bufs=4) as opool,
            tc.tile_pool(name="offp", bufs=2) as offpool,
            tc.tile_pool(name="bop", bufs=1) as bopool,
            tc.tile_pool(name="pblk", bufs=6, space="PSUM") as pblk,
            tc.tile_pool(name="poff", bufs=2, space="PSUM") as poff,
        ):
            u_t = singles.tile([P, P], F32R)
            nc.sync.dma_start(out=u_t[:], in_=u128[:])
            step_t = singles.tile([P, NBLK * NBLK], F32R)
            nc.sync.dma_start(out=step_t[:], in_=stepm[:])
            ones2_t = singles.tile([2, P], F32R)
            nc.sync.dma_start(out=ones2_t[:], in_=ones2[:])
            recip_t = singles.tile([P, NBLK], F32)
            nc.sync.dma_start(out=recip_t[:], in_=recip[:])

            for b in range(BS):
                xs = x[b].rearrange("(j p) c -> p j c", p=P)   # [128, 16, 512]
                ys = y[b].rearrange("(j p) c -> p j c", p=P)

                xhs, xls = [], []
                for h in range(NQ):
                    xt = xpool.tile([P, HALF], F32, tag="xt")
                    xt3 = xt.rearrange("p (j c) -> p j c", c=C)
                    nc.sync.dma_start(out=xt3[:], in_=xs[:, h * NH:(h + 1) * NH, :])
                    xh = xhpool.tile([P, HALF], F32R, tag="xh")
                    nc.scalar.copy(out=xh[:], in_=xt[:])
                    xl = xlpool.tile([P, HALF], F32R, tag="xl")
                    nc.vector.tensor_sub(out=xl[:], in0=xt[:], in1=xh[:].bitcast(F32))
                    xhs.append(xh)
                    xls.append(xl)

                # off[m, c] = sum_{k<m} (block-k column sum), one PSUM bank
                offp_t = poff.tile([NBLK, C], F32)
                for k in range(NBLK):
                    sel = step_t[:, k * NBLK:(k + 1) * NBLK]
                    for part, src in ((0, xhs), (1, xls)):
                        nc.tensor.matmul(
                            offp_t[:],
                            sel,
                            src[k // NH][:, (k % NH) * C:(k % NH + 1) * C],
                            start=(k == 0 and part == 0),
                            stop=(k == NBLK - 1 and part == 1),
                        )
                off_hi = offpool.tile([NBLK, C], F32R, tag="offhi")
                nc.scalar.copy(out=off_hi[:], in_=offp_t[:])
                off_lo = offpool.tile([NBLK, C], F32R, tag="offlo")
                nc.vector.tensor_sub(
                    out=off_lo[:], in0=offp_t[:], in1=off_hi[:].bitcast(F32)
                )

                # scatter offset rows to partitions 0/1 of the staging tile:
                # bo[0, j*C:(j+1)*C] = off_hi[j], bo[1, ...] = off_lo[j]
                bo = bopool.tile([2, NBLK * C], F32R)
                bo3 = bo.rearrange("p (j c) -> p j c", c=C)
                nc.sync.dma_start(out=bo3[0:1, :, :], in_=off_hi[:])
                nc.sync.dma_start(out=bo3[1:2, :, :], in_=off_lo[:])

                # main scan: every matmul's weights are either U or ones2;
                # evictions all on DVE (ACT reads PSUM at ~half DVE's rate,
                # stretching the window in which PE matmuls contend with
                # eviction reads for PSUM bandwidth)
                for h in range(NQ):
                    ot = opool.tile([P, HALF], F32, tag="ot")
                    for jj in range(NH):
                        j = h * NH + jj
                        cs = slice(jj * C, (jj + 1) * C)
                        pb = pblk.tile([P, C], F32)
                        nc.tensor.matmul(pb[:], u_t[:], xhs[h][:, cs],
                                         start=True, stop=False)
                        nc.tensor.matmul(pb[:], u_t[:], xls[h][:, cs],
                                         start=False, stop=(j == 0))
                        if j > 0:
                            nc.tensor.matmul(
                                pb[:], ones2_t[:],
                                bo[:, j * C:(j + 1) * C],
                                start=False, stop=True,
                            )
                        nc.vector.tensor_scalar_mul(
                            ot[:, cs], pb[:], recip_t[:, j:j + 1]
                        )
                    ot3 = ot.rearrange("p (j c) -> p j c", c=C)
                    nc.sync.dma_start(
                        out=ys[:, h * NH:(h + 1) * NH, :], in_=ot3[:]
                    )
    nc.finalize()
    return nc


def _consts():
    u = np.triu(np.ones((P, P), dtype=np.float32))
    step = np.zeros((P, NBLK * NBLK), dtype=np.float32)
    for k in range(NBLK):
        for m in range(NBLK):
            if m > k:
                step[:, k * NBLK + m] = 1.0
    ones2 = np.ones((2, P), dtype=np.float32)
    recip = (1.0 / np.arange(1, T + 1, dtype=np.float32)).reshape(NBLK, P).T.copy()
    return u, step, ones2, recip


def run(x, trace=False):
    x = np.ascontiguousarray(np.asarray(x, dtype=np.float32))
    assert x.shape == (B, T, C), x.shape
    if "nc" not in _cache:
        _cache["nc"] = _build()
    nc = _cache["nc"]
    u, step, ones2, recip = _consts()
    in_maps = [
        {
            "x": np.ascontiguousarray(x[i * BS:(i + 1) * BS]),
            "u128": u,
            "stepm": step,
            "ones2": ones2,
            "recip": recip,
        }
        for i in range(N_CORES)
    ]
    res = run_bass_kernel_spmd(nc, in_maps, list(range(N_CORES)), trace=trace)
    y = np.concatenate([res.results[i]["y"] for i in range(N_CORES)], axis=0)
    return y, res.exec_time_ns


def kernel(x):
    y, _ = run(x, trace=False)
    return y
